# revision 9
# baseline (speedup 1.0000x reference)
"""Bahdanau attention Trainium2 kernel.

reference:
    h     = tanh(x @ Wh^T + x @ Ws^T) = tanh(x @ (Wh+Ws)^T)
    score = h @ v                     (B, S)
    score = where(mask==0, -1e9, score)
    attn  = softmax(score, axis=1)    (B, S)
    ctx   = attn @ x                  (B, H)
    returns (ctx, attn)

Pure data-parallel over batch on 8 NeuronCores (8 batches/core).

Per core, fully pipelined per batch:
  1. mm1 on PE: Wc^T blocks stationary, host-pretransposed x^T (bf16) moving
     -> score_pre^T [o, t] in PSUM (o-chunks paired, 2 banks per group);
     tanh on ScalarE -> hT bf16.
  2. v-dot on PE: lhsT = v chunk [128,1], rhs = hT -> score row [1, 512] PSUM.
  3. softmax without max-subtraction (|score| <= ||v||_1 ~ 23, exp is safe in
     fp32): exp on ScalarE straight from PSUM; mask fold-in + sum in ONE
     fused DVE tensor_tensor_reduce; reciprocal + scale.
  4. context on DVE: attn row broadcast down 128 partitions via DRAM-bounce
     DMA, then per h-chunk one tensor_tensor_reduce against resident x^T
     -> ctx column [128,1]. No second copy of x, no PE transposes.
"""

import os
import sys

import numpy as np

for _p in ("/opt/trn_rl_repo", "/root/.axon_site/_ro/trn_rl_repo"):
    if os.path.isdir(_p) and _p not in sys.path:
        sys.path.append(_p)

import ml_dtypes  # noqa: E402

import concourse.bass as bass  # noqa: E402
import concourse.tile as tile  # noqa: E402
from concourse import bacc, mybir  # noqa: E402
from concourse.bass import ts  # noqa: E402
from concourse.bass_utils import run_bass_kernel_spmd  # noqa: E402

B, S, H = 64, 2048, 512
NCORES = 8
BPC = B // NCORES  # batches per core
KO = H // 128      # 4 chunks of 128 (both contraction and output)
NT = S // 512      # 4 token tiles of 512 per batch

F32 = mybir.dt.float32
BF16 = mybir.dt.bfloat16
BF16_NP = ml_dtypes.bfloat16
Alu = mybir.AluOpType


def _emit(tc, nc, xt, wct, vt, maskf, attn_sc, ctx_out, attn_out):
    AF = mybir.ActivationFunctionType

    with (
        tc.tile_pool(name="singles", bufs=1) as singles,
        tc.tile_pool(name="xtp", bufs=4) as xtp,
        tc.tile_pool(name="hp", bufs=4) as hp,
        tc.tile_pool(name="junkp", bufs=3) as junkp,
        tc.tile_pool(name="bcp", bufs=3) as bcp,
        tc.tile_pool(name="rowp", bufs=2) as rowp,
        tc.tile_pool(name="mm1ps", bufs=3, space="PSUM") as mm1ps,
        tc.tile_pool(name="rowps", bufs=2, space="PSUM") as rowps,
    ):
        # ---- PE warmup: ~3.5us of dummy matmuls so HAM unthrottles while
        # the first DMAs land ----
        warm = singles.tile([128, 512], BF16)
        nc.vector.memset(warm, 0.0)
        wps = rowps.tile([1, 512], F32, tag="row")
        for _ in range(20):
            nc.tensor.matmul(wps, lhsT=warm[:, 0:1], rhs=warm, start=True, stop=True)

        # ---- constants ----
        wct_sb = singles.tile([128, KO, H], BF16)  # [k_in_part, k_chunk, o]
        nc.sync.dma_start(out=wct_sb, in_=wct.rearrange("(kc p) o -> p kc o", p=128))
        vt_sb = singles.tile([128, KO], BF16)
        nc.sync.dma_start(out=vt_sb, in_=vt)
        maskf_sb = singles.tile([1, BPC * S], BF16)  # all rows on partition 0
        nc.sync.dma_start(out=maskf_sb, in_=maskf)
        ctx_sb = singles.tile([128, KO, BPC], F32)

        for b in range(BPC):
            # x^T for this batch: [k partition, k chunk, token]
            xt_t = xtp.tile([128, KO, S], BF16, tag="xt")
            nc.sync.dma_start(
                out=xt_t, in_=xt[b].rearrange("(kc p) t -> p kc t", p=128)
            )
            expd = rowp.tile([1, S], F32, tag="expd")
            for t in range(NT):
                hts = []
                for op in range(2):  # two o-pairs
                    ps = mm1ps.tile([128, 2, 512], F32, tag="mm1")
                    for oi in range(2):
                        o = op * 2 + oi
                        for k in range(KO):
                            nc.tensor.matmul(
                                ps[:, oi, :],
                                lhsT=wct_sb[:, k, ts(o, 128)],
                                rhs=xt_t[:, k, ts(t, 512)],
                                start=(k == 0),
                                stop=(k == KO - 1),
                            )
                    ht = hp.tile([128, 2, 512], BF16, tag="ht")
                    nc.scalar.activation(out=ht, in_=ps, func=AF.Tanh)
                    hts.append(ht)
                sp = rowps.tile([1, 512], F32, tag="row")
                for o in range(KO):
                    nc.tensor.matmul(
                        sp,
                        lhsT=vt_sb[:, o : o + 1],
                        rhs=hts[o // 2][:, o % 2, :],
                        start=(o == 0),
                        stop=(o == KO - 1),
                    )
                # exp straight off PSUM (no max subtraction needed)
                nc.scalar.activation(
                    out=expd[:, ts(t, 512)], in_=sp, func=AF.Exp
                )
            # masked = expd * mask ; sm = sum(masked)   (one fused DVE op)
            masked = rowp.tile([1, S], F32, tag="masked")
            sm = rowp.tile([1, 1], F32, tag="sm")
            nc.vector.scalar_tensor_tensor(
                out=masked,
                in0=expd,
                scalar=1.0,
                in1=maskf_sb[:, b * S : (b + 1) * S],
                op0=Alu.mult,
                op1=Alu.mult,
                accum_out=sm,
            )
            rc = rowp.tile([1, 1], F32, tag="rc")
            nc.vector.reciprocal(rc, sm)
            attnf = rowp.tile([1, S], F32, tag="attnf")
            nc.vector.tensor_scalar_mul(attnf, masked, rc)
            # fp32 attention weights out (scalar HWDGE ring: keeps the sync
            # ring free for x^T prefetches)
            nc.scalar.dma_start(out=attn_out[b : b + 1, :], in_=attnf)
            # bf16 staging row in DRAM (SWDGE cast), then broadcast-load
            # across all 128 partitions
            nc.gpsimd.dma_start(out=attn_sc[b : b + 1, :], in_=attnf)
            att_bc = bcp.tile([128, S], BF16, tag="bc")
            row = attn_sc[b]
            bc_ap = bass.AP(tensor=row.tensor, offset=row.offset, ap=[[0, 128]] + list(row.ap))
            nc.scalar.dma_start(out=att_bc, in_=bc_ap)
            # context: ctx[hc*128+p] = sum_t xT[p, hc, t] * attn[t]
            for hc in range(KO):
                junk = junkp.tile([128, S], BF16, tag="junk")
                nc.vector.scalar_tensor_tensor(
                    out=junk,
                    in0=xt_t[:, hc, :],
                    scalar=1.0,
                    in1=att_bc,
                    op0=Alu.mult,
                    op1=Alu.mult,
                    accum_out=ctx_sb[:, hc, b : b + 1],
                )
        ctx_out_t = ctx_out.rearrange("b (hc p) -> p hc b", p=128)
        for hc in range(KO):
            nc.scalar.dma_start(out=ctx_out_t[:, hc, :], in_=ctx_sb[:, hc, :])


def build():
    nc = bacc.Bacc(
        "TRN2", target_bir_lowering=False, debug=False, num_devices=NCORES
    )
    xt = nc.dram_tensor("xt", [BPC, H, S], BF16, kind="ExternalInput").ap()
    wct = nc.dram_tensor("wct", [H, H], BF16, kind="ExternalInput").ap()
    vt = nc.dram_tensor("vt", [128, KO], BF16, kind="ExternalInput").ap()
    maskf = nc.dram_tensor("maskf", [1, BPC * S], BF16, kind="ExternalInput").ap()
    attn_sc = nc.dram_tensor("attn_sc", [BPC, S], BF16).ap()  # internal scratch
    ctx_out = nc.dram_tensor("ctx", [BPC, H], F32, kind="ExternalOutput").ap()
    attn_out = nc.dram_tensor("attn", [BPC, S], F32, kind="ExternalOutput").ap()
    with tile.TileContext(nc) as tc:
        _emit(tc, nc, xt, wct, vt, maskf, attn_sc, ctx_out, attn_out)
    nc.compile()
    return nc


def make_in_maps(encoder_outputs, mask, Wh, Ws, v):
    x = np.ascontiguousarray(encoder_outputs, dtype=np.float32)
    xt_b = np.ascontiguousarray(x.transpose(0, 2, 1)).astype(BF16_NP)  # [B, H, S]
    wc = (np.asarray(Wh, np.float32) + np.asarray(Ws, np.float32)).T  # [k, o]
    wct_b = np.ascontiguousarray(wc).astype(BF16_NP)
    vt_b = np.ascontiguousarray(
        np.asarray(v, np.float32).reshape(KO, 128).T
    ).astype(BF16_NP)                                          # [128, KO]
    maskf = (np.asarray(mask) != 0).astype(BF16_NP)            # [B, S]

    in_maps = []
    for c in range(NCORES):
        sl = slice(c * BPC, (c + 1) * BPC)
        in_maps.append(
            {
                "xt": xt_b[sl],
                "wct": wct_b,
                "vt": vt_b,
                "maskf": maskf[sl].reshape(1, -1),
            }
        )
    return in_maps


_NC_CACHE = {}


def get_nc():
    if "nc" not in _NC_CACHE:
        _NC_CACHE["nc"] = build()
    return _NC_CACHE["nc"]


def kernel(encoder_outputs, mask, Wh, Ws, v, _trace=False, _trace_kwargs=None):
    nc = get_nc()
    in_maps = make_in_maps(encoder_outputs, mask, Wh, Ws, v)
    res = run_bass_kernel_spmd(
        nc,
        in_maps,
        core_ids=list(range(NCORES)),
        trace=_trace,
        **(_trace_kwargs or {}),
    )
    ctx = np.concatenate([r["ctx"] for r in res.results], axis=0)
    attn = np.concatenate([r["attn"] for r in res.results], axis=0)
    kernel.last_results = res
    return ctx.astype(np.float32), attn.astype(np.float32)


# revision 10
# speedup vs baseline: 1.1771x; 1.1771x over previous
"""Bahdanau attention Trainium2 kernel.

reference:
    h     = tanh(x @ Wh^T + x @ Ws^T) = tanh(x @ (Wh+Ws)^T)
    score = h @ v                     (B, S)
    score = where(mask==0, -1e9, score)
    attn  = softmax(score, axis=1)    (B, S)
    ctx   = attn @ x                  (B, H)
    returns (ctx, attn)

Pure data-parallel over batch on 8 NeuronCores (8 batches/core).

Per core, fully pipelined per batch:
  1. mm1 on PE: Wc^T blocks stationary, host-pretransposed x^T (bf16) moving
     -> score_pre^T [o, t] in PSUM (o-chunks paired, 2 banks per group);
     tanh on ScalarE -> hT bf16.
  2. v-dot on PE: lhsT = v chunk [128,1], rhs = hT -> score row [1, 512] PSUM.
  3. softmax without max-subtraction (|score| <= ||v||_1 ~ 23, exp is safe in
     fp32): exp on ScalarE straight from PSUM; mask fold-in + sum in ONE
     fused DVE tensor_tensor_reduce; reciprocal + scale.
  4. context on DVE: attn row broadcast down 128 partitions via DRAM-bounce
     DMA, then per h-chunk one tensor_tensor_reduce against resident x^T
     -> ctx column [128,1]. No second copy of x, no PE transposes.
"""

import os
import sys

import numpy as np

for _p in ("/opt/trn_rl_repo", "/root/.axon_site/_ro/trn_rl_repo"):
    if os.path.isdir(_p) and _p not in sys.path:
        sys.path.append(_p)

import ml_dtypes  # noqa: E402

import concourse.bass as bass  # noqa: E402
import concourse.tile as tile  # noqa: E402
from concourse import bacc, mybir  # noqa: E402
from concourse.bass import ts  # noqa: E402
from concourse.bass_utils import run_bass_kernel_spmd  # noqa: E402

B, S, H = 64, 2048, 512
NCORES = 8
BPC = B // NCORES  # batches per core
KO = H // 128      # 4 chunks of 128 (both contraction and output)
NT = S // 512      # 4 token tiles of 512 per batch

F32 = mybir.dt.float32
BF16 = mybir.dt.bfloat16
BF16_NP = ml_dtypes.bfloat16
Alu = mybir.AluOpType


def _emit(tc, nc, xt, wct, vt, maskf, attn_sc, ctx_out, attn_out):
    AF = mybir.ActivationFunctionType

    with (
        tc.tile_pool(name="singles", bufs=1) as singles,
        tc.tile_pool(name="xtp", bufs=4) as xtp,
        tc.tile_pool(name="hp", bufs=4) as hp,
        tc.tile_pool(name="junkp", bufs=5) as junkp,
        tc.tile_pool(name="bcp", bufs=3) as bcp,
        tc.tile_pool(name="rowp", bufs=2) as rowp,
        tc.tile_pool(name="mm1ps", bufs=3, space="PSUM") as mm1ps,
        tc.tile_pool(name="rowps", bufs=2, space="PSUM") as rowps,
    ):
        # ---- PE warmup: ~3.5us of dummy matmuls so HAM unthrottles while
        # the first DMAs land ----
        warm = singles.tile([128, 512], BF16)
        nc.vector.memset(warm, 0.0)
        wps = rowps.tile([1, 512], F32, tag="row")
        for _ in range(20):
            nc.tensor.matmul(wps, lhsT=warm[:, 0:1], rhs=warm, start=True, stop=True)

        # ---- constants ----
        wct_sb = singles.tile([128, KO, H], BF16)  # [k_in_part, k_chunk, o]
        nc.sync.dma_start(out=wct_sb, in_=wct.rearrange("(kc p) o -> p kc o", p=128))
        vt_sb = singles.tile([128, KO], BF16)
        nc.sync.dma_start(out=vt_sb, in_=vt)
        maskf_sb = singles.tile([1, BPC * S], BF16)  # all rows on partition 0
        nc.sync.dma_start(out=maskf_sb, in_=maskf)
        ctx_sb = singles.tile([128, KO, BPC], F32)

        for b in range(BPC):
            # x^T for this batch: [k partition, k chunk, token]
            xt_t = xtp.tile([128, KO, S], BF16, tag="xt")
            nc.sync.dma_start(
                out=xt_t, in_=xt[b].rearrange("(kc p) t -> p kc t", p=128)
            )
            expd = rowp.tile([1, S], F32, tag="expd")
            for t in range(NT):
                hts = []
                for op in range(2):  # two o-pairs
                    ps = mm1ps.tile([128, 2, 512], F32, tag="mm1")
                    for oi in range(2):
                        o = op * 2 + oi
                        for k in range(KO):
                            nc.tensor.matmul(
                                ps[:, oi, :],
                                lhsT=wct_sb[:, k, ts(o, 128)],
                                rhs=xt_t[:, k, ts(t, 512)],
                                start=(k == 0),
                                stop=(k == KO - 1),
                            )
                    ht = hp.tile([128, 2, 512], BF16, tag="ht")
                    nc.scalar.activation(out=ht, in_=ps, func=AF.Tanh)
                    hts.append(ht)
                sp = rowps.tile([1, 512], F32, tag="row")
                for o in range(KO):
                    nc.tensor.matmul(
                        sp,
                        lhsT=vt_sb[:, o : o + 1],
                        rhs=hts[o // 2][:, o % 2, :],
                        start=(o == 0),
                        stop=(o == KO - 1),
                    )
                # exp straight off PSUM (no max subtraction needed)
                nc.scalar.activation(
                    out=expd[:, ts(t, 512)], in_=sp, func=AF.Exp
                )
            # masked = expd * mask ; sm = sum(masked)   (one fused DVE op)
            masked = rowp.tile([1, S], F32, tag="masked")
            sm = rowp.tile([1, 1], F32, tag="sm")
            nc.vector.scalar_tensor_tensor(
                out=masked,
                in0=expd,
                scalar=1.0,
                in1=maskf_sb[:, b * S : (b + 1) * S],
                op0=Alu.mult,
                op1=Alu.mult,
                accum_out=sm,
            )
            rc = rowp.tile([1, 1], F32, tag="rc")
            nc.vector.reciprocal(rc, sm)
            attnf = rowp.tile([1, S], F32, tag="attnf")
            nc.vector.tensor_scalar_mul(attnf, masked, rc)
            # fp32 attention weights out (scalar HWDGE ring: keeps the sync
            # ring free for x^T prefetches)
            nc.gpsimd.dma_start(out=attn_out[b : b + 1, :], in_=attnf)
            # bf16 staging row in DRAM (SWDGE cast), then broadcast-load
            # across all 128 partitions
            nc.gpsimd.dma_start(out=attn_sc[b : b + 1, :], in_=attnf)
            att_bc = bcp.tile([128, S], BF16, tag="bc")
            row = attn_sc[b]
            bc_ap = bass.AP(tensor=row.tensor, offset=row.offset, ap=[[0, 128]] + list(row.ap))
            nc.gpsimd.dma_start(out=att_bc, in_=bc_ap)
            # context: ctx[hc*128+p] = sum_t xT[p, hc, t] * attn[t]
            for hc in range(KO):
                junk = junkp.tile([128, S], BF16, tag="junk")
                nc.vector.scalar_tensor_tensor(
                    out=junk,
                    in0=xt_t[:, hc, :],
                    scalar=1.0,
                    in1=att_bc,
                    op0=Alu.mult,
                    op1=Alu.mult,
                    accum_out=ctx_sb[:, hc, b : b + 1],
                )
        ctx_out_t = ctx_out.rearrange("b (hc p) -> p hc b", p=128)
        for hc in range(KO):
            nc.gpsimd.dma_start(out=ctx_out_t[:, hc, :], in_=ctx_sb[:, hc, :])


def build():
    nc = bacc.Bacc(
        "TRN2", target_bir_lowering=False, debug=False, num_devices=NCORES
    )
    xt = nc.dram_tensor("xt", [BPC, H, S], BF16, kind="ExternalInput").ap()
    wct = nc.dram_tensor("wct", [H, H], BF16, kind="ExternalInput").ap()
    vt = nc.dram_tensor("vt", [128, KO], BF16, kind="ExternalInput").ap()
    maskf = nc.dram_tensor("maskf", [1, BPC * S], BF16, kind="ExternalInput").ap()
    attn_sc = nc.dram_tensor("attn_sc", [BPC, S], BF16).ap()  # internal scratch
    ctx_out = nc.dram_tensor("ctx", [BPC, H], F32, kind="ExternalOutput").ap()
    attn_out = nc.dram_tensor("attn", [BPC, S], F32, kind="ExternalOutput").ap()
    with tile.TileContext(nc) as tc:
        _emit(tc, nc, xt, wct, vt, maskf, attn_sc, ctx_out, attn_out)
    nc.compile()
    return nc


def make_in_maps(encoder_outputs, mask, Wh, Ws, v):
    x = np.ascontiguousarray(encoder_outputs, dtype=np.float32)
    xt_b = np.ascontiguousarray(x.transpose(0, 2, 1)).astype(BF16_NP)  # [B, H, S]
    wc = (np.asarray(Wh, np.float32) + np.asarray(Ws, np.float32)).T  # [k, o]
    wct_b = np.ascontiguousarray(wc).astype(BF16_NP)
    vt_b = np.ascontiguousarray(
        np.asarray(v, np.float32).reshape(KO, 128).T
    ).astype(BF16_NP)                                          # [128, KO]
    maskf = (np.asarray(mask) != 0).astype(BF16_NP)            # [B, S]

    in_maps = []
    for c in range(NCORES):
        sl = slice(c * BPC, (c + 1) * BPC)
        in_maps.append(
            {
                "xt": xt_b[sl],
                "wct": wct_b,
                "vt": vt_b,
                "maskf": maskf[sl].reshape(1, -1),
            }
        )
    return in_maps


_NC_CACHE = {}


def get_nc():
    if "nc" not in _NC_CACHE:
        _NC_CACHE["nc"] = build()
    return _NC_CACHE["nc"]


def kernel(encoder_outputs, mask, Wh, Ws, v, _trace=False, _trace_kwargs=None):
    nc = get_nc()
    in_maps = make_in_maps(encoder_outputs, mask, Wh, Ws, v)
    res = run_bass_kernel_spmd(
        nc,
        in_maps,
        core_ids=list(range(NCORES)),
        trace=_trace,
        **(_trace_kwargs or {}),
    )
    ctx = np.concatenate([r["ctx"] for r in res.results], axis=0)
    attn = np.concatenate([r["attn"] for r in res.results], axis=0)
    kernel.last_results = res
    return ctx.astype(np.float32), attn.astype(np.float32)
